# revision 3
# baseline (speedup 1.0000x reference)
"""Multi-head causal attention (B=4, T=2048, C=1024, H=16, DH=64) on 8 TRN2 cores.

Sharding: core = (batch b, head-half). Each core computes 8 heads of batch b
and a partial output projection (its 512 rows of Wo); the host sums the two
partials per batch and adds bo.

On-chip layout is fully "transposed": matmul computes out = lhsT.T @ rhs, so
we keep x^T, q^T, k^T resident with the contraction dim on partitions.
Scores are computed as ST[s, t] = k_s . q_t (contraction d=64, two heads
row-tiled onto the 128-row PE array). Softmax runs without max-subtraction
(scores are bounded ~ +-5 for this input distribution): exp on ScalarE reads
PSUM directly, causal zeroing via gpsimd.affine_select post-exp, and the
denominator comes for free as row 64 of the AV matmul (V is augmented with a
ones column, M=65).
"""

import numpy as np

import concourse.bass as bass
import concourse.tile as tile
from concourse import bacc, mybir
import concourse.bass_utils as bass_utils

# Problem shapes (hardcoded; kernel.py must be self-contained).
H, DH, C = 16, 64, 1024
B, T = 4, 2048
N_CORES = 8
HPC = 8            # heads per core
NPAIR = HPC // 2   # head pairs per core
P = 128
CCH = C // P       # 8 contraction chunks of 128
TT = 512           # t tile width (attention + projections)
NT = T // TT       # 4
NSB = T // P       # 16 s blocks
SCALE = 1.0 / 8.0  # 1/sqrt(DH)
F32 = mybir.dt.float32

_CACHE = {}


def _build():
    """Emit the Bass/Tile program (identical for every core)."""
    from contextlib import ExitStack

    nc = bacc.Bacc("TRN2", target_bir_lowering=False, debug=False)
    xt_d = nc.dram_tensor("xt", [C, T], F32, kind="ExternalInput").ap()
    wq_d = nc.dram_tensor("wq", [C, HPC * DH], F32, kind="ExternalInput").ap()
    wk_d = nc.dram_tensor("wk", [C, HPC * DH], F32, kind="ExternalInput").ap()
    wv_d = nc.dram_tensor("wv", [C, HPC * DH], F32, kind="ExternalInput").ap()
    wo_d = nc.dram_tensor("wo", [HPC * DH, C], F32, kind="ExternalInput").ap()
    y_d = nc.dram_tensor("y", [T, C], F32, kind="ExternalOutput").ap()

    with tile.TileContext(nc) as tc, ExitStack() as ctx:
        # ---- persistent SBUF tensors (live across phases) ----
        persist = ctx.enter_context(tc.tile_pool(name="persist", bufs=1))
        qT = [persist.tile([P, T], F32, name=f"qT{p}", tag=f"qT{p}") for p in range(NPAIR)]
        kT = [persist.tile([P, T], F32, name=f"kT{p}", tag=f"kT{p}") for p in range(NPAIR)]
        # v augmented with a ones column per head: [s_chunk, head, DH+1]
        v_aug = [persist.tile([P, HPC, DH + 1], F32, name=f"va{c}", tag=f"va{c}")
                 for c in range(NSB)]
        ones1 = persist.tile([1, DH], F32, name="ones1", tag="ones1")
        nc.vector.memset(ones1, 1.0)

        # ================= Phase 1: QKV projections =================
        with tc.tile_pool(name="wqkv", bufs=1) as wpool, \
             tc.tile_pool(name="xin", bufs=2) as xpool, \
             tc.tile_pool(name="ps1", bufs=4, space="PSUM") as ps1:
            wq_s = [wpool.tile([P, HPC * DH], F32, name=f"wq{c}", tag=f"wq{c}") for c in range(CCH)]
            wk_s = [wpool.tile([P, HPC * DH], F32, name=f"wk{c}", tag=f"wk{c}") for c in range(CCH)]
            wv_s = [wpool.tile([P, HPC * DH], F32, name=f"wv{c}", tag=f"wv{c}") for c in range(CCH)]
            for c in range(CCH):
                nc.sync.dma_start(out=wq_s[c], in_=wq_d[c * P:(c + 1) * P, :])
                nc.sync.dma_start(out=wk_s[c], in_=wk_d[c * P:(c + 1) * P, :])
                nc.sync.dma_start(out=wv_s[c], in_=wv_d[c * P:(c + 1) * P, :])

            for j in range(NT):
                xt = xpool.tile([P, CCH, TT], F32, tag="xt")
                for c in range(CCH):
                    nc.sync.dma_start(
                        out=xt[:, c, :],
                        in_=xt_d[c * P:(c + 1) * P, j * TT:(j + 1) * TT])
                # q^T, k^T for each head pair: [pair_rows(128), t]
                for p in range(NPAIR):
                    for wsb, dstT in ((wq_s, qT), (wk_s, kT)):
                        ps = ps1.tile([P, TT], F32, tag="pqk")
                        for c in range(CCH):
                            nc.tensor.matmul(
                                out=ps,
                                lhsT=wsb[c][:, p * P:(p + 1) * P],
                                rhs=xt[:, c, :],
                                start=(c == 0), stop=(c == CCH - 1))
                        nc.vector.tensor_copy(
                            dstT[p][:, j * TT:(j + 1) * TT], ps)
                # v in [s, head*DH] layout, one s-block (128 rows) at a time
                for sb in range(4):
                    s_idx = j * 4 + sb
                    ps = ps1.tile([P, HPC * DH], F32, tag="pv")
                    for c in range(CCH):
                        nc.tensor.matmul(
                            out=ps,
                            lhsT=xt[:, c, sb * P:(sb + 1) * P],
                            rhs=wv_s[c],
                            start=(c == 0), stop=(c == CCH - 1))
                    nc.vector.tensor_copy(
                        v_aug[s_idx][:, :, 0:DH],
                        ps.rearrange("p (h d) -> p h d", h=HPC))
                    nc.vector.memset(v_aug[s_idx][:, :, DH:DH + 1], 1.0)

        # ============ Phase 2+3: attention + normalize ============
        oT_pool = ctx.enter_context(tc.tile_pool(name="oT", bufs=1))
        oT = [oT_pool.tile([P, T], F32, name=f"oT{p}", tag=f"oT{p}") for p in range(NPAIR)]
        wo_pool = ctx.enter_context(tc.tile_pool(name="wo", bufs=1))
        wo_s = [wo_pool.tile([P, C], F32, name=f"wo{c}", tag=f"wo{c}") for c in range(NPAIR)]
        for c in range(NPAIR):
            nc.sync.dma_start(out=wo_s[c], in_=wo_d[c * P:(c + 1) * P, :])

        with tc.tile_pool(name="st_ps", bufs=2, space="PSUM") as stp, \
             tc.tile_pool(name="po_ps", bufs=4, space="PSUM") as pop, \
             tc.tile_pool(name="est", bufs=3) as estp, \
             tc.tile_pool(name="sfx", bufs=4) as sfx:
            for p in range(NPAIR):
                for j in range(NT):
                    nchunk = 4 * j + 4  # causal: s chunks 0 .. 4j+3
                    po = [pop.tile([DH + 1, TT], F32, name=f"po{_hh}", tag="po") for _hh in range(2)]
                    for c in range(nchunk):
                        st = stp.tile([P, 2, TT], F32, tag="st")
                        for hh in range(2):
                            r0 = hh * DH
                            nc.tensor.matmul(
                                out=st[:, hh, :],
                                lhsT=kT[p][r0:r0 + DH, c * P:(c + 1) * P],
                                rhs=qT[p][r0:r0 + DH, j * TT:(j + 1) * TT],
                                start=True, stop=True)
                        est = estp.tile([P, 2, TT], F32, tag="est")
                        nc.scalar.activation(
                            est, st, mybir.ActivationFunctionType.Exp,
                            scale=SCALE)
                        if c >= 4 * j:  # diagonal-crossing chunk: zero s > t
                            k_off = c - 4 * j
                            for hh in range(2):
                                nc.gpsimd.affine_select(
                                    out=est[:, hh, :], in_=est[:, hh, :],
                                    compare_op=mybir.AluOpType.is_ge,
                                    fill=0.0, base=-(P * k_off),
                                    pattern=[[1, TT]], channel_multiplier=-1)
                        for hh in range(2):
                            h = p * 2 + hh
                            nc.tensor.matmul(
                                out=po[hh],
                                lhsT=v_aug[c][:, h, :],
                                rhs=est[:, hh, :],
                                start=(c == 0), stop=(c == nchunk - 1))
                    # normalize: rows 0..63 = unnormalized o^T, row 64 = denom
                    for hh in range(2):
                        sA = sfx.tile([DH + 1, TT], F32, tag="sA")
                        nc.vector.tensor_copy(sA, po[hh])
                        rec = sfx.tile([1, TT], F32, tag="rec")
                        nc.vector.reciprocal(rec, sA[DH:DH + 1, :])
                        bc = pop.tile([DH + 1, TT], F32, name=f"bc{hh}", tag="po")
                        nc.tensor.matmul(out=bc[0:DH, :], lhsT=ones1, rhs=rec,
                                         start=True, stop=True)
                        nc.vector.tensor_mul(
                            oT[p][hh * DH:(hh + 1) * DH, j * TT:(j + 1) * TT],
                            sA[0:DH, :], bc[0:DH, :])

        # ================= Phase 4: output projection =================
        with tc.tile_pool(name="ps4", bufs=4, space="PSUM") as ps4, \
             tc.tile_pool(name="yout", bufs=4) as ypool:
            for j2 in range(C // TT):
                for tb in range(T // P):
                    ps = ps4.tile([P, TT], F32, tag="py")
                    for c in range(NPAIR):
                        nc.tensor.matmul(
                            out=ps,
                            lhsT=oT[c][:, tb * P:(tb + 1) * P],
                            rhs=wo_s[c][:, j2 * TT:(j2 + 1) * TT],
                            start=(c == 0), stop=(c == NPAIR - 1))
                    yt = ypool.tile([P, TT], F32, tag="yt")
                    nc.vector.tensor_copy(yt, ps)
                    nc.sync.dma_start(
                        out=y_d[tb * P:(tb + 1) * P, j2 * TT:(j2 + 1) * TT],
                        in_=yt)

    nc.compile()
    return nc


def _get_nc():
    if "nc" not in _CACHE:
        _CACHE["nc"] = _build()
    return _CACHE["nc"]


def _shard(x, Wq, Wk, Wv, Wo):
    """Per-core input dicts: core = 2*b + half."""
    in_maps = []
    for core in range(N_CORES):
        b, half = divmod(core, 2)
        hs = slice(half * HPC, (half + 1) * HPC)
        # [H_c, C, DH] -> [C, H_c*DH] with column h*DH+d
        wq = np.ascontiguousarray(
            np.transpose(Wq[hs], (1, 0, 2)).reshape(C, HPC * DH))
        wk = np.ascontiguousarray(
            np.transpose(Wk[hs], (1, 0, 2)).reshape(C, HPC * DH))
        wv = np.ascontiguousarray(
            np.transpose(Wv[hs], (1, 0, 2)).reshape(C, HPC * DH))
        in_maps.append({
            "xt": np.ascontiguousarray(x[b].T),
            "wq": wq, "wk": wk, "wv": wv,
            "wo": np.ascontiguousarray(Wo[half * HPC * DH:(half + 1) * HPC * DH, :]),
        })
    return in_maps


def _run(in_maps, trace=False):
    nc = _get_nc()
    return bass_utils.run_bass_kernel_spmd(
        nc, in_maps, core_ids=list(range(N_CORES)), trace=trace)


def _gather(results, bo):
    out = np.empty((B, T, C), dtype=np.float32)
    for b in range(B):
        out[b] = results[2 * b]["y"] + results[2 * b + 1]["y"] + bo
    return out


def kernel(x, Wq, Wk, Wv, Wo, bo):
    x = np.asarray(x, dtype=np.float32)
    res = _run(_shard(x, np.asarray(Wq), np.asarray(Wk),
                      np.asarray(Wv), np.asarray(Wo)))
    return _gather(res.results, np.asarray(bo, dtype=np.float32))


def kernel_traced(x, Wq, Wk, Wv, Wo, bo):
    """Like kernel() but captures an NTFF profile; returns (out, BassKernelResults)."""
    import sys, types
    if "antenv.axon_hooks" not in sys.modules:
        mod = types.ModuleType("antenv.axon_hooks")
        _state = {"hook": None}
        mod.set_axon_ntff_profile_hook = lambda h: _state.__setitem__("hook", h)
        mod.get_axon_ntff_profile_hook = lambda: _state["hook"]
        sys.modules["antenv.axon_hooks"] = mod
        from trn_agent_boot.trn_boot import _ntff_profile_via_ctypes
        mod.set_axon_ntff_profile_hook(
            _ntff_profile_via_ctypes("/opt/axon/libaxon_pjrt.so"))
    bass_utils.upload_artifacts = lambda tmpdir: "local://" + tmpdir
    x = np.asarray(x, dtype=np.float32)
    res = _run(_shard(x, np.asarray(Wq), np.asarray(Wk),
                      np.asarray(Wv), np.asarray(Wo)), trace=True)
    return _gather(res.results, np.asarray(bo, dtype=np.float32)), res


# revision 27
# speedup vs baseline: 4.3362x; 4.3362x over previous
"""Multi-head causal attention (B=4, T=2048, C=1024, H=16, DH=64) on 8 TRN2 cores.

Sharding: core = (batch b, head-half). Each core computes 8 heads of batch b
and a partial output projection (its 512 rows of Wo); the host sums the two
partials per batch and adds bo.

On-chip layout is fully "transposed": matmul computes out = lhsT.T @ rhs, so
we keep x^T, q^T, k^T resident with the contraction dim on partitions.
Scores are computed as ST[s, t] = k_s . q_t (contraction d=64, two heads
row-tiled onto the 128-row PE array). Softmax runs without max-subtraction
(scores are bounded ~ +-5 for this input distribution): exp on ScalarE reads
PSUM directly, causal zeroing via gpsimd.affine_select post-exp, and the
denominator comes for free as row 64 of the AV matmul (V is augmented with a
ones column, M=65).
"""

import numpy as np

import concourse.bass as bass
import concourse.tile as tile
from concourse import bacc, mybir
import concourse.bass_utils as bass_utils

# Problem shapes (hardcoded; kernel.py must be self-contained).
H, DH, C = 16, 64, 1024
B, T = 4, 2048
N_CORES = 8
HPC = 8            # heads per core
NPAIR = HPC // 2   # head pairs per core
P = 128
CCH = C // P       # 8 contraction chunks of 128
TT = 512           # t tile width (attention + projections)
NT = T // TT       # 4
NSB = T // P       # 16 s blocks
SCALE = 1.0 / 8.0  # 1/sqrt(DH)
F32 = mybir.dt.float32

_CACHE = {}


def _build():
    """Emit the Bass/Tile program (identical for every core)."""
    from contextlib import ExitStack

    nc = bacc.Bacc("TRN2", target_bir_lowering=False, debug=False)
    xt_d = nc.dram_tensor("xt", [C, T], BF16, kind="ExternalInput").ap()
    wq_d = nc.dram_tensor("wq", [C, HPC * DH], BF16, kind="ExternalInput").ap()
    wk_d = nc.dram_tensor("wk", [C, HPC * DH], BF16, kind="ExternalInput").ap()
    wv_d = nc.dram_tensor("wv", [C, HPC * DH], BF16, kind="ExternalInput").ap()
    wo_d = nc.dram_tensor("wo", [HPC * DH, C], BF16, kind="ExternalInput").ap()
    y_d = nc.dram_tensor("y", [T, C], F32, kind="ExternalOutput").ap()
    # DRAM bounce rows for softmax-denominator partition-broadcast
    rb_d = nc.dram_tensor("rbounce", [NPAIR * NT * 2, TT], F32).ap()

    with tile.TileContext(nc) as tc, ExitStack() as ctx:
        # ---- persistent SBUF tensors ----
        persist = ctx.enter_context(tc.tile_pool(name="persist", bufs=1))
        ypool = ctx.enter_context(tc.tile_pool(name="yout", bufs=4))
        qT = [persist.tile([P, T], BF16, name=f"qT{p}", tag=f"qT{p}") for p in range(NPAIR)]
        kT = [persist.tile([P, T], BF16, name=f"kT{p}", tag=f"kT{p}") for p in range(NPAIR)]
        v_aug = [persist.tile([P, HPC, DH + 1], BF16, name=f"va{c}", tag=f"va{c}")
                 for c in range(NSB)]
        oT = [persist.tile([P, T], BF16, name=f"oT{p}", tag=f"oT{p}")
              for p in range(NPAIR)]
        wo_s = [persist.tile([P, C], BF16, name=f"wo{c}", tag=f"wo{c}")
                for c in range(NPAIR)]

        with tc.tile_pool(name="wqkv", bufs=1) as wpool, \
             tc.tile_pool(name="xin", bufs=1) as xpool, \
             tc.tile_pool(name="ps1", bufs=2, space="PSUM") as ps1, \
             tc.tile_pool(name="st_ps", bufs=2, space="PSUM") as stp, \
             tc.tile_pool(name="po_ps", bufs=2, space="PSUM") as pop, \
             tc.tile_pool(name="est", bufs=4) as estp, \
             tc.tile_pool(name="sfx", bufs=4) as sfx:
            # one batched DMA per weight / per x^T quarter
            wq_a = wpool.tile([P, CCH, HPC * DH], BF16, name="wq_a", tag="wq_a")
            wk_a = wpool.tile([P, CCH, HPC * DH], BF16, name="wk_a", tag="wk_a")
            wv_a = wpool.tile([P, CCH, HPC * DH], BF16, name="wv_a", tag="wv_a")
            xt = xpool.tile([P, CCH, T], BF16, tag="xt")
            for c in range(CCH):
                nc.sync.dma_start(
                    out=xt[:, c, 0:TT], in_=xt_d[c * P:(c + 1) * P, 0:TT])
            for dst, srcd in ((wv_a, wv_d), (wq_a, wq_d), (wk_a, wk_d)):
                half = CCH // 2
                nc.sync.dma_start(
                    out=dst[:, 0:half, :],
                    in_=srcd[0:half * P, :].rearrange("(c p) n -> p c n", p=P))
                nc.sync.dma_start(
                    out=dst[:, half:CCH, :],
                    in_=srcd[half * P:, :].rearrange("(c p) n -> p c n", p=P))
            for j in range(1, NT):
                for c in range(CCH):
                    nc.sync.dma_start(
                        out=xt[:, c, j * TT:(j + 1) * TT],
                        in_=xt_d[c * P:(c + 1) * P, j * TT:(j + 1) * TT])
            for c in range(NPAIR):
                nc.sync.dma_start(out=wo_s[c], in_=wo_d[c * P:(c + 1) * P, :])

            def emit_v(s_idx):
                ps = ps1.tile([P, TT], F32, tag="p1", name="psv")
                for c in range(CCH):
                    nc.tensor.matmul(
                        out=ps,
                        lhsT=xt[:, c, s_idx * P:(s_idx + 1) * P],
                        rhs=wv_a[:, c, :],
                        start=(c == 0), stop=(c == CCH - 1))
                nc.vector.tensor_copy(
                    v_aug[s_idx][:, :, 0:DH],
                    ps.rearrange("p (h d) -> p h d", h=HPC))
                nc.vector.memset(v_aug[s_idx][:, :, DH:DH + 1], 1.0)

            def emit_proj(wsb, dstT, p, j):
                ps = ps1.tile([P, TT], F32, tag="p1", name="psqk")
                for c in range(CCH):
                    nc.tensor.matmul(
                        out=ps,
                        lhsT=wsb[:, c, p * P:(p + 1) * P],
                        rhs=xt[:, c, j * TT:(j + 1) * TT],
                        start=(c == 0), stop=(c == CCH - 1))
                nc.vector.tensor_copy(
                    dstT[p][:, j * TT:(j + 1) * TT], ps)

            def emit_q(p, j):
                emit_proj(wq_a, qT, p, j)

            def emit_k(p, j):
                emit_proj(wk_a, kT, p, j)

            def emit_wo(j, pool):
                for tb in range(4 * j, 4 * j + 4):
                    # both j2 halves accumulate together so each oT lhsT is
                    # loaded once and reused for two matmuls
                    pss = [pool.tile([P, TT], F32, tag="py", name=f"psy{_j}")
                           for _j in range(C // TT)]
                    for c in range(NPAIR):
                        for j2 in range(C // TT):
                            nc.tensor.matmul(
                                out=pss[j2],
                                lhsT=oT[c][:, tb * P:(tb + 1) * P],
                                rhs=wo_s[c][:, j2 * TT:(j2 + 1) * TT],
                                start=(c == 0), stop=(c == NPAIR - 1))
                    for j2 in range(C // TT):
                        yt = ypool.tile([P, TT], F32, tag="yt")
                        if (tb + j2) % 2 == 0:
                            nc.scalar.copy(yt, pss[j2])
                        else:
                            nc.vector.tensor_copy(yt, pss[j2])
                        nc.sync.dma_start(
                            out=y_d[tb * P:(tb + 1) * P, j2 * TT:(j2 + 1) * TT],
                            in_=yt)

            def emit_attn(p, j, fills=None, prefills=None):
                nchunk = 4 * j + 4  # causal: s chunks 0 .. 4j+3
                po = [pop.tile([DH + 1, TT], F32, name=f"po{_hh}", tag="po")
                      for _hh in range(2)]
                for c in range(nchunk):
                    # diagonal-crossing chunks (c >= 4j) only have valid
                    # scores at t-columns f >= 128*(c-4j); restrict QK, exp
                    # and AV to that range (the select zeroes the rest).
                    f0 = max(0, P * (c - 4 * j))
                    st = stp.tile([P, 2, TT], F32, tag="st")
                    for hh in range(2):
                        r0 = hh * DH
                        nc.tensor.matmul(
                            out=st[:, hh, f0:TT],
                            lhsT=kT[p][r0:r0 + DH, c * P:(c + 1) * P],
                            rhs=qT[p][r0:r0 + DH, j * TT + f0:(j + 1) * TT],
                            start=True, stop=True)
                    est = estp.tile([P, 2, TT], BF16, tag="est")
                    nc.scalar.activation(
                        est[:, :, f0:TT], st[:, :, f0:TT],
                        mybir.ActivationFunctionType.Exp,
                        scale=SCALE)
                    if prefills and c in prefills:
                        for fn in prefills[c]:
                            fn()
                    if c >= 4 * j:  # zero s > t (and the un-exp'd area)
                        k_off = c - 4 * j
                        for hh in range(2):
                            nc.gpsimd.affine_select(
                                out=est[:, hh, f0:TT], in_=est[:, hh, f0:TT],
                                compare_op=mybir.AluOpType.is_ge,
                                fill=0.0, base=-(P * k_off) + f0,
                                pattern=[[1, TT - f0]], channel_multiplier=-1)
                    for hh in range(2):
                        h = p * 2 + hh
                        nc.tensor.matmul(
                            out=po[hh][:, f0:TT],
                            lhsT=v_aug[c][:, h, :],
                            rhs=est[:, hh, f0:TT],
                            start=(c == 0), stop=(c == nchunk - 1))
                    # low-priority PE fill emitted between attention chunks
                    if fills and c in fills:
                        for fn in fills[c]:
                            fn()
                # normalize: rows 0..63 = unnormalized o^T, row 64 = denom
                sAs = []
                rs2 = sfx.tile([33, TT], F32, tag="rs2")
                for hh in range(2):
                    sA = sfx.tile([DH, TT], F32, name=f"sA{hh}", tag=f"sA{hh}")
                    nc.vector.tensor_copy(sA, po[hh][0:DH, :])
                    nc.vector.tensor_copy(rs2[32 * hh:32 * hh + 1, :],
                                          po[hh][DH:DH + 1, :])
                    sAs.append(sA)
                rec = sfx.tile([33, TT], F32, tag="rec")
                rscr = sfx.tile([33, TT], F32, tag="rscr")
                # rows 1..31 are garbage; one batched reciprocal, only rows
                # 0 and 32 are consumed. approx_accurate is ~2 ULP, plenty
                # under the bf16 noise floor.
                nc.vector.reciprocal_approx_accurate(rec, rs2, rscr)
                for hh in range(2):
                    r = (p * NT + j) * 2 + hh
                    nc.sync.dma_start(out=rb_d[r:r + 1, :],
                                      in_=rec[32 * hh:32 * hh + 1, :])
                    bc = sfx.tile([DH, TT], F32, name=f"bc{hh}", tag=f"bc{hh}")
                    rb_row = rb_d[r:r + 1, :]
                    bcast = bass.AP(tensor=rb_row.tensor, offset=rb_row.offset,
                                    ap=[[0, DH]] + [list(a) for a in rb_row.ap[1:]])
                    nc.sync.dma_start(out=bc, in_=bcast)
                    nc.vector.tensor_mul(
                        oT[p][hh * DH:(hh + 1) * DH, j * TT:(j + 1) * TT],
                        sAs[hh], bc)

            # Coarse interleave: pair 0 carries the v-blocks between its
            # attention tiles; pairs 1-3 carry their own q/k tile.
            emit_q(0, 0)
            emit_k(0, 0)
            emit_attn(0, 0, prefills={
                i: [lambda s=i: emit_v(s)] for i in range(4)})
            for j in range(1, NT):
                emit_q(0, j)
                pre = {i: [lambda s=4 * j + i: emit_v(s)] for i in range(4)}
                pre[1] = pre[1] + [lambda jj=j: emit_k(0, jj)]
                emit_attn(0, j, prefills=pre)
            for p in range(1, NPAIR):
                for j in range(NT):
                    emit_q(p, j)
                    if j == 0:
                        emit_k(p, 0)
                        emit_attn(p, 0)
                    else:
                        emit_attn(p, j, prefills={
                            1: [lambda pp=p, jj=j: emit_k(pp, jj)]})

        with tc.tile_pool(name="ps4", bufs=4, space="PSUM") as ps4:
            for j in range(NT):
                emit_wo(j, ps4)

    nc.compile()
    return nc


def _get_nc():
    if "nc" not in _CACHE:
        _CACHE["nc"] = _build()
    return _CACHE["nc"]


def _shard(x, Wq, Wk, Wv, Wo):
    """Per-core input dicts: core = 2*b + half."""
    in_maps = []
    for core in range(N_CORES):
        b, half = divmod(core, 2)
        hs = slice(half * HPC, (half + 1) * HPC)
        # [H_c, C, DH] -> [C, H_c*DH] with column h*DH+d
        wq = np.ascontiguousarray(
            np.transpose(Wq[hs], (1, 0, 2)).reshape(C, HPC * DH))
        wk = np.ascontiguousarray(
            np.transpose(Wk[hs], (1, 0, 2)).reshape(C, HPC * DH))
        wv = np.ascontiguousarray(
            np.transpose(Wv[hs], (1, 0, 2)).reshape(C, HPC * DH))
        in_maps.append({
            "xt": np.ascontiguousarray(x[b].T),
            "wq": wq, "wk": wk, "wv": wv,
            "wo": np.ascontiguousarray(Wo[half * HPC * DH:(half + 1) * HPC * DH, :]),
        })
    return in_maps


def _run(in_maps, trace=False):
    nc = _get_nc()
    return bass_utils.run_bass_kernel_spmd(
        nc, in_maps, core_ids=list(range(N_CORES)), trace=trace)


def _gather(results, bo):
    out = np.empty((B, T, C), dtype=np.float32)
    for b in range(B):
        out[b] = results[2 * b]["y"] + results[2 * b + 1]["y"] + bo
    return out


def kernel(x, Wq, Wk, Wv, Wo, bo):
    x = np.asarray(x, dtype=np.float32)
    res = _run(_shard(x, np.asarray(Wq), np.asarray(Wk),
                      np.asarray(Wv), np.asarray(Wo)))
    return _gather(res.results, np.asarray(bo, dtype=np.float32))


def kernel_traced(x, Wq, Wk, Wv, Wo, bo):
    """Like kernel() but captures an NTFF profile; returns (out, BassKernelResults)."""
    import sys, types
    if "antenv.axon_hooks" not in sys.modules:
        mod = types.ModuleType("antenv.axon_hooks")
        _state = {"hook": None}
        mod.set_axon_ntff_profile_hook = lambda h: _state.__setitem__("hook", h)
        mod.get_axon_ntff_profile_hook = lambda: _state["hook"]
        sys.modules["antenv.axon_hooks"] = mod
        from trn_agent_boot.trn_boot import _ntff_profile_via_ctypes
        mod.set_axon_ntff_profile_hook(
            _ntff_profile_via_ctypes("/opt/axon/libaxon_pjrt.so"))
    bass_utils.upload_artifacts = lambda tmpdir: "local://" + tmpdir
    x = np.asarray(x, dtype=np.float32)
    res = _run(_shard(x, np.asarray(Wq), np.asarray(Wk),
                      np.asarray(Wv), np.asarray(Wo)), trace=True)
    return _gather(res.results, np.asarray(bo, dtype=np.float32)), res


# revision 29
# speedup vs baseline: 4.4167x; 1.0186x over previous
"""Multi-head causal attention (B=4, T=2048, C=1024, H=16, DH=64) on 8 TRN2 cores.

Sharding: core = (batch b, head-half). Each core computes 8 heads of batch b
and a partial output projection (its 512 rows of Wo); the host sums the two
partials per batch and adds bo.

On-chip layout is fully "transposed": matmul computes out = lhsT.T @ rhs, so
we keep x^T, q^T, k^T resident with the contraction dim on partitions.
Scores are computed as ST[s, t] = k_s . q_t (contraction d=64, two heads
row-tiled onto the 128-row PE array). Softmax runs without max-subtraction
(scores are bounded ~ +-5 for this input distribution): exp on ScalarE reads
PSUM directly, causal zeroing via gpsimd.affine_select post-exp, and the
denominator comes for free as row 64 of the AV matmul (V is augmented with a
ones column, M=65).
"""

import numpy as np

import concourse.bass as bass
import concourse.tile as tile
from concourse import bacc, mybir
import concourse.bass_utils as bass_utils

# Problem shapes (hardcoded; kernel.py must be self-contained).
H, DH, C = 16, 64, 1024
B, T = 4, 2048
N_CORES = 8
HPC = 8            # heads per core
NPAIR = HPC // 2   # head pairs per core
P = 128
CCH = C // P       # 8 contraction chunks of 128
TT = 512           # t tile width (attention + projections)
NT = T // TT       # 4
NSB = T // P       # 16 s blocks
SCALE = 1.0 / 8.0  # 1/sqrt(DH)
F32 = mybir.dt.float32

_CACHE = {}


def _build():
    """Emit the Bass/Tile program (identical for every core)."""
    from contextlib import ExitStack

    nc = bacc.Bacc("TRN2", target_bir_lowering=False, debug=False)
    xt_d = nc.dram_tensor("xt", [C, T], BF16, kind="ExternalInput").ap()
    wq_d = nc.dram_tensor("wq", [C, HPC * DH], BF16, kind="ExternalInput").ap()
    wk_d = nc.dram_tensor("wk", [C, HPC * DH], BF16, kind="ExternalInput").ap()
    wv_d = nc.dram_tensor("wv", [C, HPC * DH], BF16, kind="ExternalInput").ap()
    wo_d = nc.dram_tensor("wo", [HPC * DH, C], BF16, kind="ExternalInput").ap()
    y_d = nc.dram_tensor("y", [T, C], F32, kind="ExternalOutput").ap()
    # DRAM bounce rows for softmax-denominator partition-broadcast
    rb_d = nc.dram_tensor("rbounce", [NPAIR * NT * 2, TT], F32).ap()

    with tile.TileContext(nc) as tc, ExitStack() as ctx:
        # ---- persistent SBUF tensors ----
        persist = ctx.enter_context(tc.tile_pool(name="persist", bufs=1))
        ypool = ctx.enter_context(tc.tile_pool(name="yout", bufs=4))
        qT = [persist.tile([P, T], BF16, name=f"qT{p}", tag=f"qT{p}") for p in range(NPAIR)]
        kT = [persist.tile([P, T], BF16, name=f"kT{p}", tag=f"kT{p}") for p in range(NPAIR)]
        v_aug = [persist.tile([P, HPC, DH + 1], BF16, name=f"va{c}", tag=f"va{c}")
                 for c in range(NSB)]
        oT = [persist.tile([P, T], BF16, name=f"oT{p}", tag=f"oT{p}")
              for p in range(NPAIR)]
        wo_s = [persist.tile([P, C], BF16, name=f"wo{c}", tag=f"wo{c}")
                for c in range(NPAIR)]

        with tc.tile_pool(name="wqkv", bufs=1) as wpool, \
             tc.tile_pool(name="xin", bufs=1) as xpool, \
             tc.tile_pool(name="ps1", bufs=2, space="PSUM") as ps1, \
             tc.tile_pool(name="st_ps", bufs=2, space="PSUM") as stp, \
             tc.tile_pool(name="po_ps", bufs=2, space="PSUM") as pop, \
             tc.tile_pool(name="est", bufs=4) as estp, \
             tc.tile_pool(name="sfx", bufs=4) as sfx:
            # one batched DMA per weight / per x^T quarter
            wq_a = wpool.tile([P, CCH, HPC * DH], BF16, name="wq_a", tag="wq_a")
            wk_a = wpool.tile([P, CCH, HPC * DH], BF16, name="wk_a", tag="wk_a")
            wv_a = wpool.tile([P, CCH, HPC * DH], BF16, name="wv_a", tag="wv_a")
            xt = xpool.tile([P, CCH, T], BF16, tag="xt")
            for c in range(CCH):
                nc.sync.dma_start(
                    out=xt[:, c, 0:TT], in_=xt_d[c * P:(c + 1) * P, 0:TT])
            for dst, srcd in ((wv_a, wv_d), (wq_a, wq_d), (wk_a, wk_d)):
                half = CCH // 2
                nc.sync.dma_start(
                    out=dst[:, 0:half, :],
                    in_=srcd[0:half * P, :].rearrange("(c p) n -> p c n", p=P))
                nc.sync.dma_start(
                    out=dst[:, half:CCH, :],
                    in_=srcd[half * P:, :].rearrange("(c p) n -> p c n", p=P))
            for j in range(1, NT):
                for c in range(CCH):
                    nc.sync.dma_start(
                        out=xt[:, c, j * TT:(j + 1) * TT],
                        in_=xt_d[c * P:(c + 1) * P, j * TT:(j + 1) * TT])
            for c in range(NPAIR):
                nc.sync.dma_start(out=wo_s[c], in_=wo_d[c * P:(c + 1) * P, :])

            # HAM warmup: keep the PE busy with throwaway matmuls while the
            # first DMAs land, so real matmuls start at 2.4GHz.
            junk = wpool.tile([P, 16], BF16, name="junk", tag="junk")
            nc.vector.memset(junk, 0.5)
            jps = ps1.tile([P, 16], F32, tag="p1", name="jps")
            for _w in range(60):
                nc.tensor.matmul(out=jps[0:16, :], lhsT=junk, rhs=junk,
                                 start=(_w == 0), stop=(_w == 59))
            nc.vector.tensor_copy(junk[0:1, :], jps[0:1, :])

            def emit_v(s_idx):
                ps = ps1.tile([P, TT], F32, tag="p1", name="psv")
                for c in range(CCH):
                    nc.tensor.matmul(
                        out=ps,
                        lhsT=xt[:, c, s_idx * P:(s_idx + 1) * P],
                        rhs=wv_a[:, c, :],
                        start=(c == 0), stop=(c == CCH - 1))
                nc.vector.tensor_copy(
                    v_aug[s_idx][:, :, 0:DH],
                    ps.rearrange("p (h d) -> p h d", h=HPC))
                nc.vector.memset(v_aug[s_idx][:, :, DH:DH + 1], 1.0)

            def emit_proj(wsb, dstT, p, j):
                ps = ps1.tile([P, TT], F32, tag="p1", name="psqk")
                for c in range(CCH):
                    nc.tensor.matmul(
                        out=ps,
                        lhsT=wsb[:, c, p * P:(p + 1) * P],
                        rhs=xt[:, c, j * TT:(j + 1) * TT],
                        start=(c == 0), stop=(c == CCH - 1))
                nc.vector.tensor_copy(
                    dstT[p][:, j * TT:(j + 1) * TT], ps)

            def emit_q(p, j):
                emit_proj(wq_a, qT, p, j)

            def emit_k(p, j):
                emit_proj(wk_a, kT, p, j)

            def emit_wo(j, pool):
                for tb in range(4 * j, 4 * j + 4):
                    # both j2 halves accumulate together so each oT lhsT is
                    # loaded once and reused for two matmuls
                    pss = [pool.tile([P, TT], F32, tag="py", name=f"psy{_j}")
                           for _j in range(C // TT)]
                    for c in range(NPAIR):
                        for j2 in range(C // TT):
                            nc.tensor.matmul(
                                out=pss[j2],
                                lhsT=oT[c][:, tb * P:(tb + 1) * P],
                                rhs=wo_s[c][:, j2 * TT:(j2 + 1) * TT],
                                start=(c == 0), stop=(c == NPAIR - 1))
                    for j2 in range(C // TT):
                        yt = ypool.tile([P, TT], F32, tag="yt")
                        if (tb + j2) % 2 == 0:
                            nc.scalar.copy(yt, pss[j2])
                        else:
                            nc.vector.tensor_copy(yt, pss[j2])
                        nc.sync.dma_start(
                            out=y_d[tb * P:(tb + 1) * P, j2 * TT:(j2 + 1) * TT],
                            in_=yt)

            def emit_attn(p, j, fills=None, prefills=None):
                nchunk = 4 * j + 4  # causal: s chunks 0 .. 4j+3
                po = [pop.tile([DH + 1, TT], F32, name=f"po{_hh}", tag="po")
                      for _hh in range(2)]
                for c in range(nchunk):
                    # diagonal-crossing chunks (c >= 4j) only have valid
                    # scores at t-columns f >= 128*(c-4j); restrict QK, exp
                    # and AV to that range (the select zeroes the rest).
                    f0 = max(0, P * (c - 4 * j))
                    st = stp.tile([P, 2, TT], F32, tag="st")
                    for hh in range(2):
                        r0 = hh * DH
                        nc.tensor.matmul(
                            out=st[:, hh, f0:TT],
                            lhsT=kT[p][r0:r0 + DH, c * P:(c + 1) * P],
                            rhs=qT[p][r0:r0 + DH, j * TT + f0:(j + 1) * TT],
                            start=True, stop=True)
                    est = estp.tile([P, 2, TT], BF16, tag="est")
                    nc.scalar.activation(
                        est[:, :, f0:TT], st[:, :, f0:TT],
                        mybir.ActivationFunctionType.Exp,
                        scale=SCALE)
                    if prefills and c in prefills:
                        for fn in prefills[c]:
                            fn()
                    if c >= 4 * j:  # zero s > t inside the diagonal strip
                        # columns >= f0+128 are fully valid: f >= 128(k+1) >
                        # p + 128k for all p < 128, so only [f0, f0+128) needs
                        # the select.
                        k_off = c - 4 * j
                        for hh in range(2):
                            nc.gpsimd.affine_select(
                                out=est[:, hh, f0:f0 + P],
                                in_=est[:, hh, f0:f0 + P],
                                compare_op=mybir.AluOpType.is_ge,
                                fill=0.0, base=-(P * k_off) + f0,
                                pattern=[[1, P]], channel_multiplier=-1)
                    for hh in range(2):
                        h = p * 2 + hh
                        nc.tensor.matmul(
                            out=po[hh][:, f0:TT],
                            lhsT=v_aug[c][:, h, :],
                            rhs=est[:, hh, f0:TT],
                            start=(c == 0), stop=(c == nchunk - 1))
                    # low-priority PE fill emitted between attention chunks
                    if fills and c in fills:
                        for fn in fills[c]:
                            fn()
                # normalize: rows 0..63 = unnormalized o^T, row 64 = denom
                sAs = []
                rs2 = sfx.tile([33, TT], F32, tag="rs2")
                for hh in range(2):
                    sA = sfx.tile([DH, TT], F32, name=f"sA{hh}", tag=f"sA{hh}")
                    nc.vector.tensor_copy(sA, po[hh][0:DH, :])
                    nc.vector.tensor_copy(rs2[32 * hh:32 * hh + 1, :],
                                          po[hh][DH:DH + 1, :])
                    sAs.append(sA)
                rec = sfx.tile([33, TT], F32, tag="rec")
                rscr = sfx.tile([33, TT], F32, tag="rscr")
                # rows 1..31 are garbage; one batched reciprocal, only rows
                # 0 and 32 are consumed. approx_accurate is ~2 ULP, plenty
                # under the bf16 noise floor.
                nc.vector.reciprocal_approx_accurate(rec, rs2, rscr)
                for hh in range(2):
                    r = (p * NT + j) * 2 + hh
                    nc.sync.dma_start(out=rb_d[r:r + 1, :],
                                      in_=rec[32 * hh:32 * hh + 1, :])
                    bc = sfx.tile([DH, TT], F32, name=f"bc{hh}", tag=f"bc{hh}")
                    rb_row = rb_d[r:r + 1, :]
                    bcast = bass.AP(tensor=rb_row.tensor, offset=rb_row.offset,
                                    ap=[[0, DH]] + [list(a) for a in rb_row.ap[1:]])
                    nc.sync.dma_start(out=bc, in_=bcast)
                    nc.vector.tensor_mul(
                        oT[p][hh * DH:(hh + 1) * DH, j * TT:(j + 1) * TT],
                        sAs[hh], bc)

            # Coarse interleave: pair 0 carries the v-blocks between its
            # attention tiles; pairs 1-3 carry their own q/k tile.
            emit_q(0, 0)
            emit_k(0, 0)
            emit_attn(0, 0, prefills={
                i: [lambda s=i: emit_v(s)] for i in range(4)})
            for j in range(1, NT):
                emit_q(0, j)
                pre = {i: [lambda s=4 * j + i: emit_v(s)] for i in range(4)}
                pre[1] = pre[1] + [lambda jj=j: emit_k(0, jj)]
                emit_attn(0, j, prefills=pre)
            for p in range(1, NPAIR):
                for j in range(NT):
                    emit_q(p, j)
                    if j == 0:
                        emit_k(p, 0)
                        emit_attn(p, 0)
                    else:
                        emit_attn(p, j, prefills={
                            1: [lambda pp=p, jj=j: emit_k(pp, jj)]})

        with tc.tile_pool(name="ps4", bufs=4, space="PSUM") as ps4:
            for j in range(NT):
                emit_wo(j, ps4)

    nc.compile()
    return nc


def _get_nc():
    if "nc" not in _CACHE:
        _CACHE["nc"] = _build()
    return _CACHE["nc"]


def _shard(x, Wq, Wk, Wv, Wo):
    """Per-core input dicts: core = 2*b + half."""
    in_maps = []
    for core in range(N_CORES):
        b, half = divmod(core, 2)
        hs = slice(half * HPC, (half + 1) * HPC)
        # [H_c, C, DH] -> [C, H_c*DH] with column h*DH+d
        wq = np.ascontiguousarray(
            np.transpose(Wq[hs], (1, 0, 2)).reshape(C, HPC * DH))
        wk = np.ascontiguousarray(
            np.transpose(Wk[hs], (1, 0, 2)).reshape(C, HPC * DH))
        wv = np.ascontiguousarray(
            np.transpose(Wv[hs], (1, 0, 2)).reshape(C, HPC * DH))
        in_maps.append({
            "xt": np.ascontiguousarray(x[b].T),
            "wq": wq, "wk": wk, "wv": wv,
            "wo": np.ascontiguousarray(Wo[half * HPC * DH:(half + 1) * HPC * DH, :]),
        })
    return in_maps


def _run(in_maps, trace=False):
    nc = _get_nc()
    return bass_utils.run_bass_kernel_spmd(
        nc, in_maps, core_ids=list(range(N_CORES)), trace=trace)


def _gather(results, bo):
    out = np.empty((B, T, C), dtype=np.float32)
    for b in range(B):
        out[b] = results[2 * b]["y"] + results[2 * b + 1]["y"] + bo
    return out


def kernel(x, Wq, Wk, Wv, Wo, bo):
    x = np.asarray(x, dtype=np.float32)
    res = _run(_shard(x, np.asarray(Wq), np.asarray(Wk),
                      np.asarray(Wv), np.asarray(Wo)))
    return _gather(res.results, np.asarray(bo, dtype=np.float32))


def kernel_traced(x, Wq, Wk, Wv, Wo, bo):
    """Like kernel() but captures an NTFF profile; returns (out, BassKernelResults)."""
    import sys, types
    if "antenv.axon_hooks" not in sys.modules:
        mod = types.ModuleType("antenv.axon_hooks")
        _state = {"hook": None}
        mod.set_axon_ntff_profile_hook = lambda h: _state.__setitem__("hook", h)
        mod.get_axon_ntff_profile_hook = lambda: _state["hook"]
        sys.modules["antenv.axon_hooks"] = mod
        from trn_agent_boot.trn_boot import _ntff_profile_via_ctypes
        mod.set_axon_ntff_profile_hook(
            _ntff_profile_via_ctypes("/opt/axon/libaxon_pjrt.so"))
    bass_utils.upload_artifacts = lambda tmpdir: "local://" + tmpdir
    x = np.asarray(x, dtype=np.float32)
    res = _run(_shard(x, np.asarray(Wq), np.asarray(Wk),
                      np.asarray(Wv), np.asarray(Wo)), trace=True)
    return _gather(res.results, np.asarray(bo, dtype=np.float32)), res
